# revision 26
# baseline (speedup 1.0000x reference)
"""Distributed causal multi-head attention for Trainium2 (8 NeuronCores).

Problem: B=2, S=2048, d_model=1024, 16 heads x 64 dims, causal softmax attention.

Strategy (tensor-parallel over heads, host-side reduction of partial outputs):
  - Each core owns 2 heads (128 of the 1024 QKV features); host pre-transposes
    x -> X^T [1024, 4096] bf16 so on-chip matmuls consume feature-on-partition
    activations directly.
  - Per core: Q^T/K^T/V^T = W^T-shard @ X^T (+bias via DVE tensor_scalar),
    attention per (batch, q-chunk) in S^T layout ([k-partitions, q-free]),
    exp on ScalarE (no max subtraction; scores are O(1)), causal masking via a
    128x128 upper-tri mask multiply on diagonal tiles, denominators via an
    appended ones column on V (M=65 attnV matmuls).
  - Scores matmuls for the two heads are row-tiled (stationary base partitions
    0/64) so they run concurrently in the PE array.
  - Emission is software-pipelined for the in-order engine queues: PE filler
    units (projection matmuls of the next row-chunk, V-transposes, output
    projection of the previous q-chunk) are pulled BETWEEN the causal mask and
    attnV of each kt step, bridging the exp latency so the PE never idles;
    DVE filler units (bias-evacs, PSUM->SBUF copies) are pulled after attnV,
    at most one heavy unit per kt, so causal masks are never queued behind
    them. ScalarE runs the softmax exp exclusively.
  - Warmup matmuls run during the initial X^T DMA to pre-warm the PE clock.
  - Output: each core writes its partial [B,S,1024] (bf16); host sums 8 cores.
"""
import os
import sys

sys.path.insert(0, "/opt/trn_rl_repo")

import numpy as np
import ml_dtypes

from concourse import bacc, mybir, tile
from concourse.bass_utils import run_bass_kernel_spmd

BF16 = mybir.dt.bfloat16
F32 = mybir.dt.float32
FP8 = mybir.dt.float8e4

B, S, DM = 2, 2048, 1024
H, DK = 16, 64
N_CORES = 8
FPC = 128           # features per core = 2 heads x 64
NKT = S // 128      # k-tiles per batch = 16
NQC = S // 512      # q-chunks per batch = 4
SCALE = 1.0 / 8.0   # 1/sqrt(64)

_cache = {}


def _build():
    nc = bacc.Bacc("TRN2", target_bir_lowering=False, debug=False, num_devices=N_CORES)

    xt = nc.dram_tensor("xt", [DM, B * S], BF16, kind="ExternalInput")
    # wpk[p, :]: [wq|wk|wv tiles (3*8*128)] + [wo (1024)] + [mask|ident (256)]
    wpk = nc.dram_tensor("wpk", [128, 3 * 8 * 128 + DM + 256], BF16, kind="ExternalInput")
    bpk = nc.dram_tensor("bpk", [FPC, 3], F32, kind="ExternalInput")
    out_ext = nc.dram_tensor("out", [B, S, DM], BF16, kind="ExternalOutput")

    EXP = mybir.ActivationFunctionType.Exp

    with tile.TileContext(nc) as tc:
        with (
            tc.tile_pool(name="xtp", bufs=1) as xtp,
            tc.tile_pool(name="wts", bufs=1) as wts,
            tc.tile_pool(name="qkv", bufs=1) as qkvp,
            tc.tile_pool(name="vnat", bufs=1) as vnatp,
            tc.tile_pool(name="work", bufs=3) as work,
            tc.tile_pool(name="stage", bufs=2) as stagep,
            tc.tile_pool(name="outp", bufs=4) as outp,
            tc.tile_pool(name="psmm", bufs=2, space="PSUM") as psmm,
            tc.tile_pool(name="psS", bufs=2, space="PSUM") as psS,
            tc.tile_pool(name="psO", bufs=1, space="PSUM") as psO,
        ):
            # ---------- load packed weights/constants (2 DMAs) ----------
            WPK_N = 3 * 8 * 128 + DM + 256
            wpk_sb = wts.tile([128, WPK_N], BF16, tag="wpk", name="wpk_sb")
            nc.sync.dma_start(wpk_sb[:], wpk[:])
            bpk_sb = wts.tile([FPC, 3], F32, tag="bpk", name="bpk_sb")
            nc.sync.dma_start(bpk_sb[:], bpk[:])

            def wslice(pr, kc):
                o = (pr * 8 + kc) * 128
                return wpk_sb[:, o:o + 128]

            wq_sb = [wslice(0, kc) for kc in range(8)]
            wk_sb = [wslice(1, kc) for kc in range(8)]
            wv_sb = [wslice(2, kc) for kc in range(8)]
            wo_sb = wpk_sb[:, 3072:3072 + DM]
            mask_sb = wpk_sb[:, 3072 + DM:3072 + DM + 128]
            ident_sb = wpk_sb[:, 3072 + DM + 128:3072 + DM + 256]
            b_ap = {"q": bpk_sb[:, 0:1], "k": bpk_sb[:, 1:2], "v": bpk_sb[:, 2:3]}

            # ---------- HAM warmup: junk matmuls while xt DMA streams in ----
            warm_ps = psmm.tile([128, 512], F32, tag="mm", name="warm_ps")
            for wi in range(10):
                nc.tensor.matmul(
                    warm_ps[:], wpk_sb[:, 0:128], wpk_sb[:, 0:512],
                    start=True, stop=True,
                )

            # ---------- xt load: rp-major pieces; first rp split for fast start
            xt_sb = []
            for kc in range(8):
                t = xtp.tile([128, B * S], BF16, tag=f"xt{kc}", name=f"xt{kc}")
                xt_sb.append(t)
            for kc in range(8):
                nc.sync.dma_start(
                    xt_sb[kc][:, 0:512], xt[kc * 128:(kc + 1) * 128, 0:512]
                )
            for kc in range(8):
                nc.sync.dma_start(
                    xt_sb[kc][:, 512:1024], xt[kc * 128:(kc + 1) * 128, 512:1024]
                )
            for rp in range(1, 4):
                for kc in range(8):
                    nc.sync.dma_start(
                        xt_sb[kc][:, rp * 1024:(rp + 1) * 1024],
                        xt[kc * 128:(kc + 1) * 128, rp * 1024:(rp + 1) * 1024],
                    )

            # ---------- persistent SBUF for Q/K/V^T -------------------------
            proj_sb = {}
            for name in ("q", "k", "v"):
                proj_sb[name] = qkvp.tile(
                    [128, B * S], BF16, tag=f"{name}T", name=f"{name}T"
                )
            qT, kT, vT = proj_sb["q"], proj_sb["k"], proj_sb["v"]
            w_by_name = {"q": wq_sb, "k": wk_sb, "v": wv_sb}
            v_nat = [[None] * NKT for _ in range(B)]

            # ---------- work generators: yield ("pe"|"dve", closure) --------
            def proj_work(rc):
                """Projection matmuls + bias-evac for row chunk rc (512 toks)."""
                for name in ("q", "k", "v"):
                    ps_box = [None]
                    def mk_mm(name, rc, k0, ps_box=ps_box):
                        def mm2():
                            if ps_box[0] is None:
                                ps_box[0] = psmm.tile(
                                    [128, 512], F32, tag="mm", name=f"ps_{name}{rc}")
                            ps = ps_box[0]
                            for kc in (k0, k0 + 1):
                                nc.tensor.matmul(
                                    ps[:], w_by_name[name][kc],
                                    xt_sb[kc][:, rc * 512:(rc + 1) * 512],
                                    start=(kc == 0), stop=(kc == 7),
                                )
                        return mm2
                    for k0 in (0, 2, 4, 6):
                        yield "pe", mk_mm(name, rc, k0)
                    def evac(name=name, rc=rc, ps_box=ps_box):
                        nc.vector.tensor_scalar_add(
                            proj_sb[name][:, rc * 512:(rc + 1) * 512],
                            ps_box[0][:], b_ap[name],
                        )
                    yield "dve", evac

            def vtrans_work(rc):
                """V natural (+ones cols) for row chunk rc's 4 k-tiles."""
                b, qc = (0, rc) if rc < 4 else (1, rc - 4)
                for kt in range(4 * qc, 4 * qc + 4):
                    ps_box = [None]
                    def vt_pe(b=b, kt=kt, ps_box=ps_box):
                        ps_box[0] = psmm.tile([128, 128], BF16, tag="mm",
                                              name=f"pst{b}_{kt}")
                        nc.tensor.transpose(
                            ps_box[0][:],
                            vT[:, b * S + kt * 128: b * S + (kt + 1) * 128],
                            ident_sb,
                        )
                    yield "pe", vt_pe
                    def vt_dve(b=b, kt=kt, ps_box=ps_box):
                        ps = ps_box[0]
                        vn = vnatp.tile([128, 130], BF16, tag=f"vn{b}_{kt}",
                                        name=f"vn{b}_{kt}")
                        nc.vector.tensor_copy(vn[:, 0:64], ps[:, 0:64])
                        nc.vector.tensor_copy(vn[:, 65:129], ps[:, 64:128])
                        nc.vector.memset(vn[:, 64:65], 1.0)
                        nc.vector.memset(vn[:, 129:130], 1.0)
                        v_nat[b][kt] = vn
                    yield "dve", vt_dve

            def outproj_work(b, qc, ot):
                """Output projection for staged ot [128 feat, 512 tok]."""
                for rt in range(4):
                    o_box = [None]
                    for nc_i in range(2):
                        ps_box = [None]
                        def op_mm(b=b, qc=qc, ot=ot, rt=rt, nc_i=nc_i,
                                  ps_box=ps_box, o_box=o_box):
                            if o_box[0] is None:
                                o_box[0] = outp.tile([128, DM], BF16, tag="osb",
                                                     name=f"osb{b}_{qc}_{rt}")
                            ps_box[0] = psmm.tile([128, 512], F32, tag="mm",
                                                  name=f"pso{b}_{qc}_{rt}_{nc_i}")
                            nc.tensor.matmul(
                                ps_box[0][:], ot[:, rt * 128:(rt + 1) * 128],
                                wo_sb[:, nc_i * 512:(nc_i + 1) * 512],
                                start=True, stop=True,
                            )
                        yield "pe", op_mm
                        def op_cp(b=b, qc=qc, rt=rt, nc_i=nc_i,
                                  ps_box=ps_box, o_box=o_box):
                            o_sb = o_box[0]
                            nc.vector.tensor_copy(
                                o_sb[:, nc_i * 512:(nc_i + 1) * 512], ps_box[0][:])
                            if nc_i == 1:
                                nc.sync.dma_start(
                                    out_ext[b, qc * 512 + rt * 128:
                                            qc * 512 + (rt + 1) * 128, :],
                                    o_sb[:],
                                )
                        yield "dve", op_cp

            # ---------- attention with engine-aware filler interleave -------
            def attention(rc, fillers):
                b, qc = (0, rc) if rc < 4 else (1, rc - 4)
                q_base = b * S + qc * 512
                nkt = 4 * qc + 4

                pe_q = []    # pending PE filler units: (seq, fn)
                dve_q = []   # pending DVE filler units: (pe_before, fn)
                state = {"yielded_pe": 0, "done_pe": 0}

                def refill():
                    while len(pe_q) < 4 and len(dve_q) < 8:
                        item = next(fillers, None)
                        if item is None:
                            return False
                        if item[0] == "pe":
                            state["yielded_pe"] += 1
                            pe_q.append(item[1])
                        else:
                            dve_q.append((state["yielded_pe"], item[1]))
                    return True

                def pull_pe(n):
                    refill()
                    for _ in range(min(n, len(pe_q))):
                        pe_q.pop(0)()
                        state["done_pe"] += 1

                def pull_dve(n):
                    refill()
                    for _ in range(min(n, len(dve_q))):
                        pe_before, fn = dve_q[0]
                        # run prerequisite PE units first (keeps pairs ordered)
                        while state["done_pe"] < pe_before and pe_q:
                            pe_q.pop(0)()
                            state["done_pe"] += 1
                        if state["done_pe"] < pe_before:
                            return
                        dve_q.pop(0)
                        fn()

                o_ps = [
                    psO.tile([65, 512], F32, tag=f"o{h}", name=f"o_ps{h}_{b}_{qc}")
                    for h in (0, 1)
                ]

                def emit_s(kt):
                    d = 128 * (kt - 4 * qc)
                    lo = max(0, d)
                    k_sl = slice(b * S + kt * 128, b * S + (kt + 1) * 128)
                    s_ps = psS.tile([128, 1024], F32, tag="s", name=f"s_{b}_{qc}_{kt}")
                    q_lo = slice(q_base + lo, q_base + 512)
                    for h in (0, 1):
                        hp = slice(64 * h, 64 * h + 64)
                        nc.tensor.matmul(
                            s_ps[:, 512 * h + lo:512 * h + 512],
                            kT[hp, k_sl], qT[hp, q_lo],
                            start=True, stop=True,
                        )
                    # additive causal mask (-300 above diagonal) applied to the
                    # scores PSUM on the scores->exp edge, which has a full
                    # pipeline period of slack -- instead of a multiplicative
                    # mask on the exp->attnV critical edge
                    if d >= 0:
                        hi = min(512, d + 128)
                        for h in (0, 1):
                            nc.vector.tensor_add(
                                s_ps[:, 512 * h + lo:512 * h + hi],
                                s_ps[:, 512 * h + lo:512 * h + hi],
                                mask_sb[:, 0:hi - lo],
                            )
                    return s_ps, lo, d

                s_cur = emit_s(0)
                for kt in range(nkt):
                    s_ps, lo, d = s_cur
                    s_nxt = emit_s(kt + 1) if kt + 1 < nkt else None
                    p_sb = work.tile([128, 1024], BF16, tag="p", name=f"p_{b}_{qc}_{kt}")
                    nc.scalar.activation(
                        p_sb[:, lo:1024], s_ps[:, lo:1024], EXP, scale=SCALE,
                    )
                    # PE fillers bridge the exp latency before attnV
                    pull_pe(1)
                    for h in (0, 1):
                        nc.tensor.matmul(
                            o_ps[h][:, lo:512],
                            v_nat[b][kt][:, 65 * h:65 * h + 65],
                            p_sb[:, 512 * h + lo:512 * h + 512],
                            start=(kt == 0), stop=(kt == nkt - 1),
                        )
                    # DVE fillers sit between masks, at most 1 heavy unit
                    pull_dve(1)
                    s_cur = s_nxt

                # normalize (per-head denominator on psum row 64) + stage
                ot = stagep.tile([128, 512], BF16, tag=f"ot{b}_{qc}", name=f"ot{b}_{qc}")
                for h in (0, 1):
                    rc_sb = work.tile([128, 512], F32, tag="recip", name=f"rcp{b}_{qc}_{h}")
                    nc.vector.tensor_copy(rc_sb[64:65, :], o_ps[h][64:65, :])
                    nc.vector.tensor_copy(rc_sb[0:1, :], rc_sb[64:65, :])
                    nc.vector.reciprocal_approx_fast(rc_sb[0:1, :], rc_sb[0:1, :])
                    nc.gpsimd.partition_broadcast(
                        rc_sb[0:64, :], rc_sb[0:1, :], channels=64
                    )
                    nc.vector.tensor_mul(
                        ot[64 * h:64 * h + 64, :], o_ps[h][0:64, :], rc_sb[0:64, :]
                    )
                # drain remaining fillers
                while True:
                    refill()
                    if not pe_q and not dve_q:
                        break
                    if pe_q:
                        pull_pe(len(pe_q))
                    if dve_q:
                        pull_dve(len(dve_q))
                return b, qc, ot

            # ---------- main pipeline ---------------------------------------
            # proj/vtrans run as dense sequential phases (PE-bound, keeps HAM
            # warm); only the previous q-chunk's output projection is
            # interleaved into the attention loop (its MMs fill small PE
            # bubbles, its copies go to the underutilized DVE).
            prev_out = None
            for rc in range(8):
                for kind, f in proj_work(rc):
                    f()
                for kind, f in vtrans_work(rc):
                    f()
                gens = []
                if prev_out is not None:
                    gens.append(outproj_work(*prev_out))

                def chain_iters(its):
                    for it in its:
                        yield from it

                prev_out = attention(rc, chain_iters(gens))

            for kind, f in outproj_work(*prev_out):
                f()

    nc.compile()
    return nc


def kernel(x, Wq, bq, Wk, bk, Wv, bv, Wo):
    if "nc" not in _cache:
        _cache["nc"] = _build()
    nc = _cache["nc"]

    bf = ml_dtypes.bfloat16
    xt = np.ascontiguousarray(np.asarray(x, np.float32).reshape(B * S, DM).T).astype(bf)
    wo_f = np.asarray(Wo, np.float32)
    # additive causal mask: 0 where q >= k (keep), -300 where q < k
    trimask = (np.triu(np.ones((128, 128), np.float32)) - 1.0) * 300.0
    ident = np.eye(128, dtype=np.float32)

    in_maps = []
    for c in range(N_CORES):
        sl = slice(c * FPC, (c + 1) * FPC)
        wpk = np.empty((128, 3 * 8 * 128 + DM + 256), np.float32)
        for pr, W in enumerate((Wq, Wk, Wv)):
            Wc = np.asarray(W, np.float32)[:, sl]          # [1024, 128]
            wpk[:, pr * 1024:(pr + 1) * 1024] = (
                Wc.reshape(8, 128, 128).transpose(1, 0, 2).reshape(128, 1024)
            )
        wpk[:, 3072:3072 + DM] = wo_f[sl, :]
        wpk[:, 3072 + DM:3072 + DM + 128] = trimask
        wpk[:, 3072 + DM + 128:] = ident
        bpk = np.stack(
            [np.asarray(b, np.float32)[sl] for b in (bq, bk, bv)], axis=1
        )
        in_maps.append({
            "xt": xt,
            "wpk": np.ascontiguousarray(wpk).astype(bf),
            "bpk": np.ascontiguousarray(bpk),
        })

    trace = bool(int(os.environ.get("ATTN_KERNEL_TRACE", "0")))
    res = run_bass_kernel_spmd(nc, in_maps, core_ids=list(range(N_CORES)), trace=trace)
    if trace:
        print(f"HW exec time: {res.exec_time_ns} ns")
        _cache["exec_time_ns"] = res.exec_time_ns
        _cache["res"] = res

    out = np.asarray(res.results[0]["out"]).astype(np.float32)
    for c in range(1, N_CORES):
        out += np.asarray(res.results[c]["out"]).astype(np.float32)
    return out


# revision 27
# speedup vs baseline: 1.0266x; 1.0266x over previous
"""Distributed causal multi-head attention for Trainium2 (8 NeuronCores).

Problem: B=2, S=2048, d_model=1024, 16 heads x 64 dims, causal softmax attention.

Strategy (tensor-parallel over heads, host-side reduction of partial outputs):
  - Each core owns 2 heads (128 of the 1024 QKV features); host pre-transposes
    x -> X^T [1024, 4096] bf16 so on-chip matmuls consume feature-on-partition
    activations directly.
  - Per core: Q^T/K^T/V^T = W^T-shard @ X^T (+bias via DVE tensor_scalar),
    attention per (batch, q-chunk) in S^T layout ([k-partitions, q-free]),
    exp on ScalarE (no max subtraction; scores are O(1)), causal masking via a
    128x128 upper-tri mask multiply on diagonal tiles, denominators via an
    appended ones column on V (M=65 attnV matmuls).
  - Scores matmuls for the two heads are row-tiled (stationary base partitions
    0/64) so they run concurrently in the PE array.
  - Emission is software-pipelined for the in-order engine queues: PE filler
    units (projection matmuls of the next row-chunk, V-transposes, output
    projection of the previous q-chunk) are pulled BETWEEN the causal mask and
    attnV of each kt step, bridging the exp latency so the PE never idles;
    DVE filler units (bias-evacs, PSUM->SBUF copies) are pulled after attnV,
    at most one heavy unit per kt, so causal masks are never queued behind
    them. ScalarE runs the softmax exp exclusively.
  - Warmup matmuls run during the initial X^T DMA to pre-warm the PE clock.
  - Output: each core writes its partial [B,S,1024] (bf16); host sums 8 cores.
"""
import os
import sys

sys.path.insert(0, "/opt/trn_rl_repo")

import numpy as np
import ml_dtypes

from concourse import bacc, mybir, tile
from concourse.bass_utils import run_bass_kernel_spmd

BF16 = mybir.dt.bfloat16
F32 = mybir.dt.float32
FP8 = mybir.dt.float8e4

B, S, DM = 2, 2048, 1024
H, DK = 16, 64
N_CORES = 8
FPC = 128           # features per core = 2 heads x 64
NKT = S // 128      # k-tiles per batch = 16
NQC = S // 512      # q-chunks per batch = 4
SCALE = 1.0 / 8.0   # 1/sqrt(64)

_cache = {}


def _build():
    nc = bacc.Bacc("TRN2", target_bir_lowering=False, debug=False, num_devices=N_CORES)

    xt = nc.dram_tensor("xt", [DM, B * S], BF16, kind="ExternalInput")
    # wpk[p, :]: [wq|wk|wv tiles (3*8*128)] + [wo (1024)] + [mask|ident (256)]
    wpk = nc.dram_tensor("wpk", [128, 3 * 8 * 128 + DM + 256], BF16, kind="ExternalInput")
    bpk = nc.dram_tensor("bpk", [FPC, 3], F32, kind="ExternalInput")
    out_ext = nc.dram_tensor("out", [B, S, DM], BF16, kind="ExternalOutput")

    EXP = mybir.ActivationFunctionType.Exp

    with tile.TileContext(nc) as tc:
        with (
            tc.tile_pool(name="xtp", bufs=1) as xtp,
            tc.tile_pool(name="wts", bufs=1) as wts,
            tc.tile_pool(name="qkv", bufs=1) as qkvp,
            tc.tile_pool(name="vnat", bufs=1) as vnatp,
            tc.tile_pool(name="work", bufs=3) as work,
            tc.tile_pool(name="stage", bufs=2) as stagep,
            tc.tile_pool(name="outp", bufs=4) as outp,
            tc.tile_pool(name="psmm", bufs=2, space="PSUM") as psmm,
            tc.tile_pool(name="psS", bufs=2, space="PSUM") as psS,
            tc.tile_pool(name="psO", bufs=1, space="PSUM") as psO,
        ):
            # ---------- load packed weights/constants (2 DMAs) ----------
            WPK_N = 3 * 8 * 128 + DM + 256
            wpk_sb = wts.tile([128, WPK_N], BF16, tag="wpk", name="wpk_sb")
            nc.sync.dma_start(wpk_sb[:], wpk[:])
            bpk_sb = wts.tile([FPC, 3], F32, tag="bpk", name="bpk_sb")
            nc.sync.dma_start(bpk_sb[:], bpk[:])

            def wslice(pr, kc):
                o = (pr * 8 + kc) * 128
                return wpk_sb[:, o:o + 128]

            wq_sb = [wslice(0, kc) for kc in range(8)]
            wk_sb = [wslice(1, kc) for kc in range(8)]
            wv_sb = [wslice(2, kc) for kc in range(8)]
            wo_sb = wpk_sb[:, 3072:3072 + DM]
            mask_sb = wpk_sb[:, 3072 + DM:3072 + DM + 128]
            ident_sb = wpk_sb[:, 3072 + DM + 128:3072 + DM + 256]
            b_ap = {"q": bpk_sb[:, 0:1], "k": bpk_sb[:, 1:2], "v": bpk_sb[:, 2:3]}

            # ---------- HAM warmup: junk matmuls while xt DMA streams in ----
            warm_ps = psmm.tile([128, 512], F32, tag="mm", name="warm_ps")
            for wi in range(10):
                nc.tensor.matmul(
                    warm_ps[:], wpk_sb[:, 0:128], wpk_sb[:, 0:512],
                    start=True, stop=True,
                )

            # ---------- xt load: rp-major pieces; first rp split for fast start
            xt_sb = []
            for kc in range(8):
                t = xtp.tile([128, B * S], BF16, tag=f"xt{kc}", name=f"xt{kc}")
                xt_sb.append(t)
            for kc in range(8):
                nc.sync.dma_start(
                    xt_sb[kc][:, 0:512], xt[kc * 128:(kc + 1) * 128, 0:512]
                )
            for kc in range(8):
                nc.sync.dma_start(
                    xt_sb[kc][:, 512:1024], xt[kc * 128:(kc + 1) * 128, 512:1024]
                )
            for rp in range(1, 4):
                for kc in range(8):
                    nc.sync.dma_start(
                        xt_sb[kc][:, rp * 1024:(rp + 1) * 1024],
                        xt[kc * 128:(kc + 1) * 128, rp * 1024:(rp + 1) * 1024],
                    )

            # ---------- persistent SBUF for Q/K/V^T -------------------------
            proj_sb = {}
            for name in ("q", "k", "v"):
                proj_sb[name] = qkvp.tile(
                    [128, B * S], BF16, tag=f"{name}T", name=f"{name}T"
                )
            qT, kT, vT = proj_sb["q"], proj_sb["k"], proj_sb["v"]
            w_by_name = {"q": wq_sb, "k": wk_sb, "v": wv_sb}
            v_nat = [[None] * NKT for _ in range(B)]

            # ---------- work generators: yield ("pe"|"dve", closure) --------
            def proj_work(rc):
                """Projection matmuls + bias-evac for row chunk rc (512 toks)."""
                for name in ("q", "k", "v"):
                    ps_box = [None]
                    def mk_mm(name, rc, k0, ps_box=ps_box):
                        def mm2():
                            if ps_box[0] is None:
                                ps_box[0] = psmm.tile(
                                    [128, 512], F32, tag="mm", name=f"ps_{name}{rc}")
                            ps = ps_box[0]
                            for kc in (k0, k0 + 1):
                                nc.tensor.matmul(
                                    ps[:], w_by_name[name][kc],
                                    xt_sb[kc][:, rc * 512:(rc + 1) * 512],
                                    start=(kc == 0), stop=(kc == 7),
                                )
                        return mm2
                    for k0 in (0, 2, 4, 6):
                        yield "pe", mk_mm(name, rc, k0)
                    def evac(name=name, rc=rc, ps_box=ps_box):
                        nc.vector.tensor_scalar_add(
                            proj_sb[name][:, rc * 512:(rc + 1) * 512],
                            ps_box[0][:], b_ap[name],
                        )
                    yield "dve", evac

            def vtrans_work(rc):
                """V natural (+ones cols) for row chunk rc's 4 k-tiles."""
                b, qc = (0, rc) if rc < 4 else (1, rc - 4)
                for kt in range(4 * qc, 4 * qc + 4):
                    ps_box = [None]
                    def vt_pe(b=b, kt=kt, ps_box=ps_box):
                        ps_box[0] = psmm.tile([128, 128], BF16, tag="mm",
                                              name=f"pst{b}_{kt}")
                        nc.tensor.transpose(
                            ps_box[0][:],
                            vT[:, b * S + kt * 128: b * S + (kt + 1) * 128],
                            ident_sb,
                        )
                    yield "pe", vt_pe
                    def vt_dve(b=b, kt=kt, ps_box=ps_box):
                        ps = ps_box[0]
                        vn = vnatp.tile([128, 130], BF16, tag=f"vn{b}_{kt}",
                                        name=f"vn{b}_{kt}")
                        nc.vector.tensor_copy(vn[:, 0:64], ps[:, 0:64])
                        nc.vector.tensor_copy(vn[:, 65:129], ps[:, 64:128])
                        nc.vector.memset(vn[:, 64:65], 1.0)
                        nc.vector.memset(vn[:, 129:130], 1.0)
                        v_nat[b][kt] = vn
                    yield "dve", vt_dve

            def outproj_work(b, qc, ot):
                """Output projection for staged ot [128 feat, 512 tok]."""
                for rt in range(4):
                    o_box = [None]
                    for nc_i in range(2):
                        ps_box = [None]
                        def op_mm(b=b, qc=qc, ot=ot, rt=rt, nc_i=nc_i,
                                  ps_box=ps_box, o_box=o_box):
                            if o_box[0] is None:
                                o_box[0] = outp.tile([128, DM], BF16, tag="osb",
                                                     name=f"osb{b}_{qc}_{rt}")
                            ps_box[0] = psmm.tile([128, 512], F32, tag="mm",
                                                  name=f"pso{b}_{qc}_{rt}_{nc_i}")
                            nc.tensor.matmul(
                                ps_box[0][:], ot[:, rt * 128:(rt + 1) * 128],
                                wo_sb[:, nc_i * 512:(nc_i + 1) * 512],
                                start=True, stop=True,
                            )
                        yield "pe", op_mm
                        def op_cp(b=b, qc=qc, rt=rt, nc_i=nc_i,
                                  ps_box=ps_box, o_box=o_box):
                            o_sb = o_box[0]
                            nc.vector.tensor_copy(
                                o_sb[:, nc_i * 512:(nc_i + 1) * 512], ps_box[0][:])
                            if nc_i == 1:
                                nc.sync.dma_start(
                                    out_ext[b, qc * 512 + rt * 128:
                                            qc * 512 + (rt + 1) * 128, :],
                                    o_sb[:],
                                )
                        yield "dve", op_cp

            # ---------- attention with engine-aware filler interleave -------
            def attention(rc, fillers):
                b, qc = (0, rc) if rc < 4 else (1, rc - 4)
                q_base = b * S + qc * 512
                nkt = 4 * qc + 4

                pe_q = []    # pending PE filler units: (seq, fn)
                dve_q = []   # pending DVE filler units: (pe_before, fn)
                state = {"yielded_pe": 0, "done_pe": 0}

                def refill():
                    while len(pe_q) < 4 and len(dve_q) < 8:
                        item = next(fillers, None)
                        if item is None:
                            return False
                        if item[0] == "pe":
                            state["yielded_pe"] += 1
                            pe_q.append(item[1])
                        else:
                            dve_q.append((state["yielded_pe"], item[1]))
                    return True

                def pull_pe(n):
                    refill()
                    for _ in range(min(n, len(pe_q))):
                        pe_q.pop(0)()
                        state["done_pe"] += 1

                def pull_dve(n):
                    refill()
                    for _ in range(min(n, len(dve_q))):
                        pe_before, fn = dve_q[0]
                        # run prerequisite PE units first (keeps pairs ordered)
                        while state["done_pe"] < pe_before and pe_q:
                            pe_q.pop(0)()
                            state["done_pe"] += 1
                        if state["done_pe"] < pe_before:
                            return
                        dve_q.pop(0)
                        fn()

                o_ps = [
                    psO.tile([65, 512], F32, tag=f"o{h}", name=f"o_ps{h}_{b}_{qc}")
                    for h in (0, 1)
                ]

                def emit_s(kt):
                    d = 128 * (kt - 4 * qc)
                    lo = max(0, d)
                    k_sl = slice(b * S + kt * 128, b * S + (kt + 1) * 128)
                    s_ps = psS.tile([128, 1024], F32, tag="s", name=f"s_{b}_{qc}_{kt}")
                    q_lo = slice(q_base + lo, q_base + 512)
                    for h in (0, 1):
                        hp = slice(64 * h, 64 * h + 64)
                        nc.tensor.matmul(
                            s_ps[:, 512 * h + lo:512 * h + 512],
                            kT[hp, k_sl], qT[hp, q_lo],
                            start=True, stop=True,
                        )
                    # additive causal mask (-300 above diagonal) applied to the
                    # scores PSUM on the scores->exp edge, which has a full
                    # pipeline period of slack -- instead of a multiplicative
                    # mask on the exp->attnV critical edge
                    if d >= 0:
                        hi = min(512, d + 128)
                        for h in (0, 1):
                            nc.vector.tensor_add(
                                s_ps[:, 512 * h + lo:512 * h + hi],
                                s_ps[:, 512 * h + lo:512 * h + hi],
                                mask_sb[:, 0:hi - lo],
                            )
                    return s_ps, lo, d

                s_cur = emit_s(0)
                for kt in range(nkt):
                    s_ps, lo, d = s_cur
                    s_nxt = emit_s(kt + 1) if kt + 1 < nkt else None
                    p_sb = work.tile([128, 1024], BF16, tag="p", name=f"p_{b}_{qc}_{kt}")
                    nc.scalar.activation(
                        p_sb[:, lo:1024], s_ps[:, lo:1024], EXP, scale=SCALE,
                    )
                    # PE fillers bridge the exp latency before attnV
                    pull_pe(2)
                    for h in (0, 1):
                        nc.tensor.matmul(
                            o_ps[h][:, lo:512],
                            v_nat[b][kt][:, 65 * h:65 * h + 65],
                            p_sb[:, 512 * h + lo:512 * h + 512],
                            start=(kt == 0), stop=(kt == nkt - 1),
                        )
                    # DVE fillers sit between masks, at most 1 heavy unit
                    pull_dve(1)
                    s_cur = s_nxt

                # normalize (per-head denominator on psum row 64) + stage
                ot = stagep.tile([128, 512], BF16, tag=f"ot{b}_{qc}", name=f"ot{b}_{qc}")
                for h in (0, 1):
                    rc_sb = work.tile([128, 512], F32, tag="recip", name=f"rcp{b}_{qc}_{h}")
                    nc.vector.tensor_copy(rc_sb[64:65, :], o_ps[h][64:65, :])
                    nc.vector.tensor_copy(rc_sb[0:1, :], rc_sb[64:65, :])
                    nc.vector.reciprocal_approx_fast(rc_sb[0:1, :], rc_sb[0:1, :])
                    nc.gpsimd.partition_broadcast(
                        rc_sb[0:64, :], rc_sb[0:1, :], channels=64
                    )
                    nc.vector.tensor_mul(
                        ot[64 * h:64 * h + 64, :], o_ps[h][0:64, :], rc_sb[0:64, :]
                    )
                # drain remaining fillers
                while True:
                    refill()
                    if not pe_q and not dve_q:
                        break
                    if pe_q:
                        pull_pe(len(pe_q))
                    if dve_q:
                        pull_dve(len(dve_q))
                return b, qc, ot

            # ---------- main pipeline ---------------------------------------
            # proj/vtrans run as dense sequential phases (PE-bound, keeps HAM
            # warm); only the previous q-chunk's output projection is
            # interleaved into the attention loop (its MMs fill small PE
            # bubbles, its copies go to the underutilized DVE).
            prev_out = None
            for rc in range(8):
                for kind, f in proj_work(rc):
                    f()
                for kind, f in vtrans_work(rc):
                    f()
                gens = []
                if prev_out is not None:
                    gens.append(outproj_work(*prev_out))

                def chain_iters(its):
                    for it in its:
                        yield from it

                prev_out = attention(rc, chain_iters(gens))

            for kind, f in outproj_work(*prev_out):
                f()

    nc.compile()
    return nc


def kernel(x, Wq, bq, Wk, bk, Wv, bv, Wo):
    if "nc" not in _cache:
        _cache["nc"] = _build()
    nc = _cache["nc"]

    bf = ml_dtypes.bfloat16
    xt = np.ascontiguousarray(np.asarray(x, np.float32).reshape(B * S, DM).T).astype(bf)
    wo_f = np.asarray(Wo, np.float32)
    # additive causal mask: 0 where q >= k (keep), -300 where q < k
    trimask = (np.triu(np.ones((128, 128), np.float32)) - 1.0) * 300.0
    ident = np.eye(128, dtype=np.float32)

    in_maps = []
    for c in range(N_CORES):
        sl = slice(c * FPC, (c + 1) * FPC)
        wpk = np.empty((128, 3 * 8 * 128 + DM + 256), np.float32)
        for pr, W in enumerate((Wq, Wk, Wv)):
            Wc = np.asarray(W, np.float32)[:, sl]          # [1024, 128]
            wpk[:, pr * 1024:(pr + 1) * 1024] = (
                Wc.reshape(8, 128, 128).transpose(1, 0, 2).reshape(128, 1024)
            )
        wpk[:, 3072:3072 + DM] = wo_f[sl, :]
        wpk[:, 3072 + DM:3072 + DM + 128] = trimask
        wpk[:, 3072 + DM + 128:] = ident
        bpk = np.stack(
            [np.asarray(b, np.float32)[sl] for b in (bq, bk, bv)], axis=1
        )
        in_maps.append({
            "xt": xt,
            "wpk": np.ascontiguousarray(wpk).astype(bf),
            "bpk": np.ascontiguousarray(bpk),
        })

    trace = bool(int(os.environ.get("ATTN_KERNEL_TRACE", "0")))
    res = run_bass_kernel_spmd(nc, in_maps, core_ids=list(range(N_CORES)), trace=trace)
    if trace:
        print(f"HW exec time: {res.exec_time_ns} ns")
        _cache["exec_time_ns"] = res.exec_time_ns
        _cache["res"] = res

    out = np.asarray(res.results[0]["out"]).astype(np.float32)
    for c in range(1, N_CORES):
        out += np.asarray(res.results[c]["out"]).astype(np.float32)
    return out
